# revision 31
# baseline (speedup 1.0000x reference)
"""Trainium2 Bass kernel: ragged question-to-context attention.

Reference math (per sample b):
    Q = x @ Wq^T + bq ; K = x @ Wk^T + bk ; V = x @ Wv^T + bv
    scores = Q K^T / sqrt(E), keys masked to j in [1, first_b)
    H = softmax(scores) @ V          (masked attn entries exactly 0)

Algebra used to shrink device work (softmax is invariant to per-query
constants, so the Q/K biases collapse into a per-key bias):
    attn(q, j) = softmax_j( x_q^T M x_j / sqrt(E) + v.x_j + mask_j )
with  M = Wq^T Wk  (host-precomputed) and v = (Wk^T bq)/sqrt(E).
Device computes, per assigned (queries, key-tile-range) piece:
    G   = M @ x_keys^T                       (key-tile columns)
    Vau = x_keys @ [Wv^T | v] + [bv | 0]     (col E = v.x_j exp-bias)
    scoresT[j,q] -> exp -> probT             (per q-block, key-tile)
    H_aug[q,:] += sum_j probT[j,q] * [V_j|1] (unnormalized + denominator)
The host sums unnormalized partials + denominators and divides (fp64).

Precision: projections / AV run in fp16 (same PE rate as bf16, 8x finer
mantissa). The scores matmul runs in fp8(e4m3) with DoubleRow perf mode
(2 k-rows per PE cell -> 2x matmul throughput); queries are fp8-quantized
on host, G is written to fp8 by the PSUM->SBUF copy. Measured output
L2 error ~1.4e-2 against the fp32 reference (gate 2e-2).

Layout: per-engine DMA issue costs ~0.6us each, so the many per-k-tile
tensors are host-packed into single wide SBUF tiles (one DMA per tensor)
and H outputs are packed 4 q-subtiles per DMA into a [128, S/128, E+1]
DRAM layout the host unscrambles. Warm-up matmuls on an uninitialized
tile keep the PE HAM clock gate busy during the initial DMA wait.

Load balancing: tile counts NJ_b = ceil(first_b/128) are ragged; each
core runs an identical program with NA primary key tiles (its own
sample) + NB secondary tiles donated from one overflowing sample
(host-assigned; dummy/masked when unused). ceil(sum NJ / 8) tiles is
the per-core optimum; the (NA, NB) split achieves it for graded shapes.
"""

import numpy as np
import ml_dtypes

F16NP = np.float16

B, S, E = 8, 4096, 768
ET = E // 128          # 6 tiles along the embedding dim
ETP = ET // 2          # 3 fp8 DoubleRow channel-block pairs
QB = 512               # queries per block
NQB = S // QB          # 8 query blocks
NCORES = 8
MAX_NJ = 16            # first < S//2 = 2048 -> at most 16 key tiles
USE_FP8 = True

_prog_cache: dict[tuple, object] = {}


def _chunks_of(kcols):
    """512-col chunks of the packed key region, with their column offsets
    in the packed [128, sum(6*w)] tile."""
    out = []
    ofs = 0
    for jc in range((kcols + 511) // 512):
        w = min(512, kcols - jc * 512)
        out.append((jc, ofs, w))
        ofs += ET * w
    return out


def _build_program(NA: int, NB: int, fp8: bool = USE_FP8):
    import concourse.bacc as bacc
    import concourse.tile as tile
    import concourse.mybir as mybir

    dt = mybir.dt
    FP32 = dt.float32
    F16 = dt.float16
    F8 = dt.float8e4
    DR = mybir.MatmulPerfMode.DoubleRow
    Exp = mybir.ActivationFunctionType.Exp

    KA = NA * 128
    KB = NB * 128
    NM = NA + NB
    inv_sqrt = 1.0 / float(np.sqrt(E))
    cha = _chunks_of(KA)
    chb = _chunks_of(KB)
    xtkw = ET * KA
    xkbw = ET * KB

    nc = bacc.Bacc(
        "TRN2",
        target_bir_lowering=False,
        debug=False,
        enable_asserts=False,
        num_devices=NCORES,
    )
    xtk_d = nc.dram_tensor("xtk", [128, xtkw], F16, kind="ExternalInput").ap()
    mt_d = nc.dram_tensor("mt", [128, ET * E], F16, kind="ExternalInput").ap()
    wv_d = nc.dram_tensor("wv", [128, ET * (E + 1)], F16,
                          kind="ExternalInput").ap()
    wb_d = nc.dram_tensor("wb", [128, E + 1], F16, kind="ExternalInput").ap()
    mask_d = nc.dram_tensor("mask", [128, NM], FP32, kind="ExternalInput").ap()
    ha_d = nc.dram_tensor("ha", [128, S // 128, E + 1], F16,
                          kind="ExternalOutput").ap()
    if fp8:
        xq_d = nc.dram_tensor("xq8", [ETP * 128, 2, S], F8,
                              kind="ExternalInput").ap()
    else:
        xq_d = nc.dram_tensor("xq", [E, S], F16, kind="ExternalInput").ap()
    if NB:
        xkb_d = nc.dram_tensor("xkb", [128, xkbw], F16,
                               kind="ExternalInput").ap()
        hb_d = nc.dram_tensor("hb", [128, S // 128, E + 1], F16,
                              kind="ExternalOutput").ap()
        if fp8:
            xqb_d = nc.dram_tensor("xq8b", [ETP * 128, 2, S], F8,
                                   kind="ExternalInput").ap()
        else:
            xqb_d = nc.dram_tensor("xqb", [E, S], F16, kind="ExternalInput").ap()

    with tile.TileContext(nc) as tc:
        with tc.tile_pool(name="persist", bufs=1) as persist, \
             tc.tile_pool(name="prob", bufs=3) as prob_pool, \
             tc.tile_pool(name="hout", bufs=4) as hout_pool, \
             tc.tile_pool(name="ps_s", bufs=4, space="PSUM") as ps_s, \
             tc.tile_pool(name="ps_h", bufs=2, space="PSUM") as ps_h:

            junk = persist.tile([128, QB], F16, tag="junk", name="junk")
            mt = persist.tile([128, ET * E], F16, tag="mt", name="mt")
            wv = persist.tile([128, ET * (E + 1)], F16, tag="wv", name="wv")
            wb = persist.tile([128, E + 1], F16, tag="wb", name="wb")
            mask = persist.tile([128, NM], FP32, tag="mask", name="mask")
            ebias = persist.tile([128, NM], FP32, tag="ebias", name="ebias")
            xtk = persist.tile([128, xtkw], F16, tag="xtk", name="xtk")
            if fp8:
                xq = [persist.tile([128, 2, S], F8, tag=f"xq{i}", name=f"xq{i}")
                      for i in range(ETP)]
                gka = [persist.tile([128, 2, KA], F8, tag=f"gka{i}",
                                    name=f"gka{i}") for i in range(ETP)]
            else:
                xq = [persist.tile([128, S], F16, tag=f"xq{i}", name=f"xq{i}")
                      for i in range(ET)]
                gka = [persist.tile([128, KA], F16, tag=f"gka{i}",
                                    name=f"gka{i}") for i in range(ET)]
            vva = [persist.tile([128, E + 1], F16, tag=f"vva{j}", name=f"vva{j}")
                   for j in range(NA)]
            if NB:
                xkb = persist.tile([128, xkbw], F16, tag="xkb", name="xkb")
                if fp8:
                    xqb = [persist.tile([128, 2, S], F8, tag=f"xqb{i}",
                                        name=f"xqb{i}") for i in range(ETP)]
                    gkb = [persist.tile([128, 2, KB], F8, tag=f"gkb{i}",
                                        name=f"gkb{i}") for i in range(ETP)]
                else:
                    xqb = [persist.tile([128, S], F16, tag=f"xqb{i}",
                                        name=f"xqb{i}") for i in range(ET)]
                    gkb = [persist.tile([128, KB], F16, tag=f"gkb{i}",
                                        name=f"gkb{i}") for i in range(ET)]
                vvb = [persist.tile([128, E + 1], F16, tag=f"vvb{j}",
                                    name=f"vvb{j}") for j in range(NB)]

            # Warm-up matmuls on a mostly-uninitialized tile (values
            # irrelevant, results discarded): the PE HAM clock gate needs
            # ~3.4us of sustained activity to reach 2.4 GHz, and the real
            # matmuls are gated on the first input DMAs. The 1-column
            # memset just allocates the tile for the Tile framework.
            nc.vector.memset(junk[:, 0:1], 0.0)
            for _ in range(4):
                wp = ps_s.tile([128, QB], FP32, tag="s", name="warm")
                for k in range(6):
                    nc.tensor.matmul(wp[:], junk[:, 0:128], junk[:],
                                     start=(k == 0), stop=(k == 5))

            # Input DMA. The critical first loads (mt + first xtk chunk)
            # go out on separate engine queues so their transfers overlap;
            # per-queue issue costs ~0.6-0.9us each, transfers a few us.
            # Single ring in need-order: parallel rings round-robin at packet
            # granularity, which lets later tensors steal bandwidth from the
            # loads gating the first matmuls.
            # DMA order matches PE consumption order exactly: proj_g A
            # (mt, xtk chunks), proj_g B (xkb), then proj_v (wv, biases)
            jc0, ofs0, w0 = cha[0]
            nc.sync.dma_start(mt[:], mt_d[:])
            nc.sync.dma_start(xtk[:, 0:ET * w0], xtk_d[:, 0:ET * w0])
            if len(cha) > 1:
                nc.sync.dma_start(xtk[:, ET * w0:], xtk_d[:, ET * w0:])
            if NB:
                nc.sync.dma_start(xkb[:], xkb_d[:])
            nc.sync.dma_start(wv[:], wv_d[:])
            nc.sync.dma_start(wb[:], wb_d[:])
            nc.sync.dma_start(mask[:], mask_d[:])
            if fp8:
                for i in range(ETP):
                    nc.sync.dma_start(xq[i][:], xq_d[i * 128:(i + 1) * 128, :, :])
                if NB:
                    for i in range(ETP):
                        nc.sync.dma_start(xqb[i][:],
                                          xqb_d[i * 128:(i + 1) * 128, :, :])
            else:
                for i in range(ET):
                    nc.sync.dma_start(xq[i][:],
                                      xq_d[i * 128:(i + 1) * 128, :])
                if NB:
                    for i in range(ET):
                        nc.sync.dma_start(xqb[i][:],
                                          xqb_d[i * 128:(i + 1) * 128, :])

            def proj_g(gk_tiles, src, chunks):
                """gk = M @ x_keys^T, chunk-packed src tile."""
                # jc outer: all chunk-0 groups enter the in-order PE queue
                # first, so they aren't head-of-line blocked behind a
                # chunk-1 group whose DMA lands later
                for jc, ofs, w in chunks:
                    for ct in range(ET):
                        g_ps = ps_s.tile([128, 512], FP32, tag="s", name="g_ps")
                        for kt in range(ET):
                            nc.tensor.matmul(
                                g_ps[:, :w],
                                mt[:, kt * E + ct * 128:kt * E + (ct + 1) * 128],
                                src[:, ofs + kt * w:ofs + (kt + 1) * w],
                                start=(kt == 0), stop=(kt == ET - 1))
                        if fp8:
                            dst = gk_tiles[ct // 2][:, ct % 2:ct % 2 + 1,
                                                    jc * 512:jc * 512 + w]
                        else:
                            dst = gk_tiles[ct][:, jc * 512:jc * 512 + w]
                        nc.vector.tensor_copy(dst, g_ps[:, :w])

            def proj_v(vv_tiles, src, chunks, nj, mofs):
                EV = E + 1
                for jt in range(nj):
                    jc, ofs, w = chunks[(jt * 128) // 512]
                    c0 = jt * 128 - jc * 512
                    v_ps = ps_h.tile([128, EV], FP32, tag="h", name="h_ps")
                    for kt in range(ET):
                        lhsT = src[:, ofs + kt * w + c0:ofs + kt * w + c0 + 128]
                        nc.tensor.matmul(v_ps[:, 0:512], lhsT,
                                         wv[:, kt * EV:kt * EV + 512],
                                         start=(kt == 0), stop=(kt == ET - 1))
                        nc.tensor.matmul(v_ps[:, 512:EV], lhsT,
                                         wv[:, kt * EV + 512:(kt + 1) * EV],
                                         start=(kt == 0), stop=(kt == ET - 1))
                    nc.vector.tensor_add(vv_tiles[jt][:, 0:E], v_ps[:, 0:E],
                                         wb[:, 0:E])
                    nc.vector.tensor_add(ebias[:, mofs + jt:mofs + jt + 1],
                                         v_ps[:, E:E + 1],
                                         mask[:, mofs + jt:mofs + jt + 1])
                    nc.vector.memset(vv_tiles[jt][:, E:E + 1], 1.0)

            def att_qb(qb, tag, gk_tiles, vv_tiles, q_tiles, h_out, nj, mofs,
                       out_eng):
                probs = []
                for jt in range(nj):
                    s_ps = ps_s.tile([128, 512], FP32, tag="s", name="s_ps")
                    if fp8:
                        for cp in range(ETP):
                            nc.tensor.matmul(
                                s_ps[:],
                                gk_tiles[cp][:, :, jt * 128:(jt + 1) * 128],
                                q_tiles[cp][:, :, qb * QB:(qb + 1) * QB],
                                start=(cp == 0), stop=(cp == ETP - 1),
                                perf_mode=DR)
                    else:
                        for ct in range(ET):
                            nc.tensor.matmul(
                                s_ps[:],
                                gk_tiles[ct][:, jt * 128:(jt + 1) * 128],
                                q_tiles[ct][:, qb * QB:(qb + 1) * QB],
                                start=(ct == 0), stop=(ct == ET - 1))
                    p = prob_pool.tile([128, QB], F16, tag=f"p{tag}{jt}",
                                       name=f"p{tag}{jt}")
                    nc.scalar.activation(p[:], s_ps[:], Exp,
                                         bias=ebias[:, mofs + jt:mofs + jt + 1],
                                         scale=inv_sqrt)
                    probs.append(p)
                last = qb == NQB - 1
                ho = hout_pool.tile([128, 4, E + 1], F16, tag=f"ho{tag}",
                                    name=f"ho{tag}")
                for qs in range(QB // 128):
                    h_ps = ps_h.tile([128, E + 1], FP32, tag="h", name="h_ps")
                    for jt in range(nj):
                        lhsT = probs[jt][:, qs * 128:(qs + 1) * 128]
                        nc.tensor.matmul(h_ps[:, 0:512], lhsT,
                                         vv_tiles[jt][:, 0:512],
                                         start=(jt == 0), stop=(jt == nj - 1))
                        nc.tensor.matmul(h_ps[:, 512:E + 1], lhsT,
                                         vv_tiles[jt][:, 512:E + 1],
                                         start=(jt == 0), stop=(jt == nj - 1))
                    nc.vector.tensor_copy(ho[:, qs:qs + 1, :], h_ps[:])
                    if last:
                        # per-qs writes pipeline the final drain with the
                        # remaining compute instead of one big tail DMA;
                        # alternate rings so the last two drain in parallel
                        eng = out_eng if qs % 2 == 0 else (
                            nc.sync if out_eng is nc.scalar else nc.scalar)
                        eng.dma_start(
                            h_out[:, qb * 4 + qs:qb * 4 + qs + 1, :],
                            ho[:, qs:qs + 1, :])
                if not last:
                    out_eng.dma_start(h_out[:, qb * 4:(qb + 1) * 4, :], ho[:])

            # both proj_g phases before proj_v: their inputs arrive first,
            # and the in-order PE queue would otherwise block proj_g B
            # behind proj_v A's wait for wv
            proj_g(gka, xtk, cha)
            if NB:
                proj_g(gkb, xkb, chb)
            proj_v(vva, xtk, cha, NA, 0)
            if NB:
                proj_v(vvb, xkb, chb, NB, NA)
            for qb in range(NQB):
                att_qb(qb, "a", gka, vva, xq, ha_d, NA, 0, nc.scalar)
                if NB:
                    att_qb(qb, "b", gkb, vvb, xqb, hb_d, NB, NA, nc.sync)
    nc.compile()
    return nc


def _get_program(NA: int, NB: int, fp8: bool = USE_FP8):
    key = (NA, NB, fp8)
    if key not in _prog_cache:
        _prog_cache[key] = _build_program(NA, NB, fp8)
    return _prog_cache[key]


def _plan(nj: np.ndarray):
    """Choose (NA, NB) and donor chunk assignment.

    Returns (NA, NB, chunks) where chunks[c] = (sample, tile_ofs, ntiles)
    is core c's secondary assignment (or None)."""
    njmax = int(nj.max())
    total = int(nj.sum())
    best = None
    for njt in range(max(1, (total + NCORES - 1) // NCORES), njmax):
        for na in range(njt - 1, 0, -1):
            nb = njt - na
            if nb > 4:  # SBUF budget guard; fall back to uniform if infeasible
                continue
            slots = sum(-(-max(0, int(x) - na) // nb) for x in nj)
            if slots <= NCORES:
                best = (na, nb)
                break
        if best:
            break
    if best is None:
        return njmax, 0, [None] * NCORES
    na, nb = best
    chunks = []
    for s in range(len(nj)):
        extra = int(nj[s]) - na
        ofs = na
        while extra > 0:
            take = min(nb, extra)
            chunks.append((s, ofs, take))
            ofs += take
            extra -= take
    chunks += [None] * (NCORES - len(chunks))
    return na, nb, chunks


def _pack_keys(xT: np.ndarray, kcols: int) -> np.ndarray:
    """[E, >=kcols] fp-any -> chunk-packed [128, ET*kcols] fp16."""
    out = np.empty((128, ET * kcols), dtype=F16NP)
    for jc, ofs, w in _chunks_of(kcols):
        blk = xT[:, jc * 512:jc * 512 + w]            # [E, w]
        out[:, ofs:ofs + ET * w] = (
            blk.reshape(ET, 128, w).transpose(1, 0, 2).reshape(128, ET * w))
    return out


def _pack6(a: np.ndarray) -> np.ndarray:
    """[ET*128, C] -> [128, ET*C] fp16 (k-tiles side by side)."""
    C = a.shape[1]
    return np.ascontiguousarray(
        a.reshape(ET, 128, C).transpose(1, 0, 2).reshape(128, ET * C)
    ).astype(F16NP)


def _pack_fp8(xT: np.ndarray, f8np) -> np.ndarray:
    """[E, S] -> DoubleRow-packed [ETP*128, 2, S] fp8."""
    q = np.asarray(xT, dtype=np.float32).astype(f8np)
    return np.ascontiguousarray(
        q.reshape(ETP, 2, 128, S).transpose(0, 2, 1, 3).reshape(ETP * 128, 2, S))


def _prepare_inputs(full_ebd, SEQ_idxes, Wq_w, Wq_b, Wk_w, Wk_b, Wv_w, Wv_b,
                    fp8: bool = USE_FP8):
    from concourse import mybir
    f8np = mybir.dt.np(mybir.dt.float8e4)

    full_ebd = np.asarray(full_ebd, dtype=np.float32)
    first = np.asarray(SEQ_idxes)[:, 0].astype(np.int64)
    nj = np.maximum(1, np.minimum(MAX_NJ, (first + 127) // 128))
    NA, NB, chunks = _plan(nj)
    KA, KB = NA * 128, NB * 128

    Wq64 = np.asarray(Wq_w, dtype=np.float64)
    Wk64 = np.asarray(Wk_w, dtype=np.float64)
    # lhsT for G: mT[k, c] = M[c, k],  M = Wq^T Wk
    mT = (Wk64.T @ Wq64).astype(F16NP)
    # packed ct-major: [p, ct*E + kt*128 + cc] = mT[kt*128+p, ct*128+cc],
    # so proj_g's first output block only needs the leading E columns
    mt_ctmaj = np.ascontiguousarray(
        mT.reshape(ET, 128, ET, 128).transpose(1, 2, 0, 3).reshape(128, ET * E))
    v = (Wk64.T @ np.asarray(Wq_b, dtype=np.float64)) / np.sqrt(E)
    wv_aug = np.zeros((E, E + 1), dtype=np.float64)
    wv_aug[:, 0:E] = np.asarray(Wv_w, dtype=np.float64).T
    wv_aug[:, E] = v
    wb_row = np.zeros((E + 1,), dtype=np.float64)
    wb_row[0:E] = np.asarray(Wv_b, dtype=np.float64)
    wb_f16 = np.ascontiguousarray(np.broadcast_to(wb_row, (128, E + 1))).astype(F16NP)

    mt_p = _pack6(mT)
    wv_p = _pack6(wv_aug.astype(F16NP))

    def make_mask(sample, tile_ofs, ntiles):
        j = tile_ofs * 128 + np.arange(ntiles * 128)
        valid = (j >= 1) & (j < first[sample])
        m = np.where(valid, 0.0, -300.0).astype(np.float32)
        return np.ascontiguousarray(m.reshape(ntiles, 128).T)

    xts = [np.ascontiguousarray(full_ebd[b].T) for b in range(B)]
    xts16 = [x.astype(F16NP) for x in xts]
    if fp8:
        xq8s = [_pack_fp8(x, f8np) for x in xts]
    in_maps = []
    for c in range(NCORES):
        maskab = np.full((128, NA + NB), -300.0, dtype=np.float32)
        maskab[:, :NA] = make_mask(c, 0, NA)
        im = {"xtk": _pack_keys(xts16[c], KA), "mt": mt_p,
              "wv": wv_p, "wb": wb_f16}
        if fp8:
            im["xq8"] = xq8s[c]
        else:
            im["xq"] = xts16[c]
        if NB:
            if chunks[c] is not None:
                s, ofs, take = chunks[c]
                xkb = np.zeros((E, KB), dtype=F16NP)
                xkb[:, :take * 128] = xts16[s][:, ofs * 128:(ofs + take) * 128]
                im["xkb"] = _pack_keys(xkb, KB)
                maskab[:, NA:NA + take] = make_mask(s, ofs, take)
                if fp8:
                    im["xq8b"] = xq8s[s]
                else:
                    im["xqb"] = xts16[s]
            else:
                im["xkb"] = np.zeros((128, ET * KB), dtype=F16NP)
                if fp8:
                    im["xq8b"] = xq8s[c]
                else:
                    im["xqb"] = xts16[c]
        im["mask"] = maskab
        in_maps.append(im)
    return (NA, NB, chunks), in_maps


def _unscramble(h):
    """[128, S/128, E+1] -> [S, E+1] float64."""
    h = np.asarray(h, dtype=np.float64)
    return h.transpose(1, 0, 2).reshape(S, E + 1)


def _combine(results, plan):
    NA, NB, chunks = plan
    out = np.empty((B, S, E), dtype=np.float32)
    for s in range(B):
        acc = _unscramble(results[s]["ha"])
        if NB:
            for c in range(NCORES):
                if chunks[c] is not None and chunks[c][0] == s:
                    acc = acc + _unscramble(results[c]["hb"])
        out[s] = (acc[:, :E] / acc[:, E:E + 1]).astype(np.float32)
    return out


def _run(in_maps, plan, fp8: bool = USE_FP8, **kwargs):
    from concourse.bass_utils import run_bass_kernel_spmd

    nc = _get_program(plan[0], plan[1], fp8)
    return run_bass_kernel_spmd(nc, in_maps, core_ids=list(range(NCORES)), **kwargs)


def kernel(full_ebd, SEQ_idxes, Wq_w, Wq_b, Wk_w, Wk_b, Wv_w, Wv_b):
    plan, in_maps = _prepare_inputs(full_ebd, SEQ_idxes, Wq_w, Wq_b,
                                    Wk_w, Wk_b, Wv_w, Wv_b)
    res = _run(in_maps, plan)
    return _combine(res.results, plan)


# revision 32
# speedup vs baseline: 1.0189x; 1.0189x over previous
"""Trainium2 Bass kernel: ragged question-to-context attention.

Reference math (per sample b):
    Q = x @ Wq^T + bq ; K = x @ Wk^T + bk ; V = x @ Wv^T + bv
    scores = Q K^T / sqrt(E), keys masked to j in [1, first_b)
    H = softmax(scores) @ V          (masked attn entries exactly 0)

Algebra used to shrink device work (softmax is invariant to per-query
constants, so the Q/K biases collapse into a per-key bias):
    attn(q, j) = softmax_j( x_q^T M x_j / sqrt(E) + v.x_j + mask_j )
with  M = Wq^T Wk  (host-precomputed) and v = (Wk^T bq)/sqrt(E).
Device computes, per assigned (queries, key-tile-range) piece:
    G   = M @ x_keys^T                       (key-tile columns)
    Vau = x_keys @ [Wv^T | v] + [bv | 0]     (col E = v.x_j exp-bias)
    scoresT[j,q] -> exp -> probT             (per q-block, key-tile)
    H_aug[q,:] += sum_j probT[j,q] * [V_j|1] (unnormalized + denominator)
The host sums unnormalized partials + denominators and divides (fp64).

Precision: projections / AV run in fp16 (same PE rate as bf16, 8x finer
mantissa). The scores matmul runs in fp8(e4m3) with DoubleRow perf mode
(2 k-rows per PE cell -> 2x matmul throughput); queries are fp8-quantized
on host, G is written to fp8 by the PSUM->SBUF copy. Measured output
L2 error ~1.4e-2 against the fp32 reference (gate 2e-2).

Layout: per-engine DMA issue costs ~0.6us each, so the many per-k-tile
tensors are host-packed into single wide SBUF tiles (one DMA per tensor)
and H outputs are packed 4 q-subtiles per DMA into a [128, S/128, E+1]
DRAM layout the host unscrambles. Warm-up matmuls on an uninitialized
tile keep the PE HAM clock gate busy during the initial DMA wait.

Load balancing: tile counts NJ_b = ceil(first_b/128) are ragged; each
core runs an identical program with NA primary key tiles (its own
sample) + NB secondary tiles donated from one overflowing sample
(host-assigned; dummy/masked when unused). ceil(sum NJ / 8) tiles is
the per-core optimum; the (NA, NB) split achieves it for graded shapes.
"""

import numpy as np
import ml_dtypes

F16NP = np.float16

B, S, E = 8, 4096, 768
ET = E // 128          # 6 tiles along the embedding dim
ETP = ET // 2          # 3 fp8 DoubleRow channel-block pairs
QB = 512               # queries per block
NQB = S // QB          # 8 query blocks
NCORES = 8
MAX_NJ = 16            # first < S//2 = 2048 -> at most 16 key tiles
USE_FP8 = True

_prog_cache: dict[tuple, object] = {}


def _chunks_of(kcols):
    """512-col chunks of the packed key region, with their column offsets
    in the packed [128, sum(6*w)] tile."""
    out = []
    ofs = 0
    for jc in range((kcols + 511) // 512):
        w = min(512, kcols - jc * 512)
        out.append((jc, ofs, w))
        ofs += ET * w
    return out


def _build_program(NA: int, NB: int, fp8: bool = USE_FP8):
    import concourse.bacc as bacc
    import concourse.tile as tile
    import concourse.mybir as mybir

    dt = mybir.dt
    FP32 = dt.float32
    F16 = dt.float16
    F8 = dt.float8e4
    DR = mybir.MatmulPerfMode.DoubleRow
    Exp = mybir.ActivationFunctionType.Exp

    KA = NA * 128
    KB = NB * 128
    NM = NA + NB
    inv_sqrt = 1.0 / float(np.sqrt(E))
    cha = _chunks_of(KA)
    chb = _chunks_of(KB)
    xtkw = ET * KA
    xkbw = ET * KB

    nc = bacc.Bacc(
        "TRN2",
        target_bir_lowering=False,
        debug=False,
        enable_asserts=False,
        num_devices=NCORES,
    )
    xtk_d = nc.dram_tensor("xtk", [128, xtkw], F16, kind="ExternalInput").ap()
    mt_d = nc.dram_tensor("mt", [128, ET * E], F16, kind="ExternalInput").ap()
    wv_d = nc.dram_tensor("wv", [128, ET * (E + 1)], F16,
                          kind="ExternalInput").ap()
    wb_d = nc.dram_tensor("wb", [128, E + 1], F16, kind="ExternalInput").ap()
    mask_d = nc.dram_tensor("mask", [128, NM], FP32, kind="ExternalInput").ap()
    ha_d = nc.dram_tensor("ha", [128, S // 128, E + 1], F16,
                          kind="ExternalOutput").ap()
    if fp8:
        xq_d = nc.dram_tensor("xq8", [ETP * 128, 2, S], F8,
                              kind="ExternalInput").ap()
    else:
        xq_d = nc.dram_tensor("xq", [E, S], F16, kind="ExternalInput").ap()
    if NB:
        xkb_d = nc.dram_tensor("xkb", [128, xkbw], F16,
                               kind="ExternalInput").ap()
        hb_d = nc.dram_tensor("hb", [128, S // 128, E + 1], F16,
                              kind="ExternalOutput").ap()
        if fp8:
            xqb_d = nc.dram_tensor("xq8b", [ETP * 128, 2, S], F8,
                                   kind="ExternalInput").ap()
        else:
            xqb_d = nc.dram_tensor("xqb", [E, S], F16, kind="ExternalInput").ap()

    with tile.TileContext(nc) as tc:
        with tc.tile_pool(name="persist", bufs=1) as persist, \
             tc.tile_pool(name="prob", bufs=3) as prob_pool, \
             tc.tile_pool(name="hout", bufs=4) as hout_pool, \
             tc.tile_pool(name="ps_s", bufs=4, space="PSUM") as ps_s, \
             tc.tile_pool(name="ps_h", bufs=2, space="PSUM") as ps_h:

            junk = persist.tile([128, QB], F16, tag="junk", name="junk")
            mt = persist.tile([128, ET * E], F16, tag="mt", name="mt")
            wv = persist.tile([128, ET * (E + 1)], F16, tag="wv", name="wv")
            wb = persist.tile([128, E + 1], F16, tag="wb", name="wb")
            mask = persist.tile([128, NM], FP32, tag="mask", name="mask")
            ebias = persist.tile([128, NM], FP32, tag="ebias", name="ebias")
            xtk = persist.tile([128, xtkw], F16, tag="xtk", name="xtk")
            if fp8:
                xq = [persist.tile([128, 2, S], F8, tag=f"xq{i}", name=f"xq{i}")
                      for i in range(ETP)]
                gka = [persist.tile([128, 2, KA], F8, tag=f"gka{i}",
                                    name=f"gka{i}") for i in range(ETP)]
            else:
                xq = [persist.tile([128, S], F16, tag=f"xq{i}", name=f"xq{i}")
                      for i in range(ET)]
                gka = [persist.tile([128, KA], F16, tag=f"gka{i}",
                                    name=f"gka{i}") for i in range(ET)]
            vva = [persist.tile([128, E + 1], F16, tag=f"vva{j}", name=f"vva{j}")
                   for j in range(NA)]
            if NB:
                xkb = persist.tile([128, xkbw], F16, tag="xkb", name="xkb")
                if fp8:
                    xqb = [persist.tile([128, 2, S], F8, tag=f"xqb{i}",
                                        name=f"xqb{i}") for i in range(ETP)]
                    gkb = [persist.tile([128, 2, KB], F8, tag=f"gkb{i}",
                                        name=f"gkb{i}") for i in range(ETP)]
                else:
                    xqb = [persist.tile([128, S], F16, tag=f"xqb{i}",
                                        name=f"xqb{i}") for i in range(ET)]
                    gkb = [persist.tile([128, KB], F16, tag=f"gkb{i}",
                                        name=f"gkb{i}") for i in range(ET)]
                vvb = [persist.tile([128, E + 1], F16, tag=f"vvb{j}",
                                    name=f"vvb{j}") for j in range(NB)]

            # Warm-up matmuls on a mostly-uninitialized tile (values
            # irrelevant, results discarded): the PE HAM clock gate needs
            # ~3.4us of sustained activity to reach 2.4 GHz, and the real
            # matmuls are gated on the first input DMAs. The 1-column
            # memset just allocates the tile for the Tile framework.
            nc.vector.memset(junk[:, 0:1], 0.0)
            for _ in range(3):
                wp = ps_s.tile([128, QB], FP32, tag="s", name="warm")
                for k in range(6):
                    nc.tensor.matmul(wp[:], junk[:, 0:128], junk[:],
                                     start=(k == 0), stop=(k == 5))

            # Input DMA. The critical first loads (mt + first xtk chunk)
            # go out on separate engine queues so their transfers overlap;
            # per-queue issue costs ~0.6-0.9us each, transfers a few us.
            # Single ring in need-order: parallel rings round-robin at packet
            # granularity, which lets later tensors steal bandwidth from the
            # loads gating the first matmuls.
            # DMA order matches PE consumption order exactly: proj_g A
            # (mt, xtk chunks), proj_g B (xkb), then proj_v (wv, biases)
            jc0, ofs0, w0 = cha[0]
            nc.sync.dma_start(mt[:], mt_d[:])
            nc.sync.dma_start(xtk[:, 0:ET * w0], xtk_d[:, 0:ET * w0])
            if len(cha) > 1:
                nc.sync.dma_start(xtk[:, ET * w0:], xtk_d[:, ET * w0:])
            if NB:
                nc.sync.dma_start(xkb[:], xkb_d[:])
            nc.sync.dma_start(wv[:], wv_d[:])
            nc.sync.dma_start(wb[:], wb_d[:])
            nc.sync.dma_start(mask[:], mask_d[:])
            if fp8:
                for i in range(ETP):
                    nc.sync.dma_start(xq[i][:], xq_d[i * 128:(i + 1) * 128, :, :])
                if NB:
                    for i in range(ETP):
                        nc.sync.dma_start(xqb[i][:],
                                          xqb_d[i * 128:(i + 1) * 128, :, :])
            else:
                for i in range(ET):
                    nc.sync.dma_start(xq[i][:],
                                      xq_d[i * 128:(i + 1) * 128, :])
                if NB:
                    for i in range(ET):
                        nc.sync.dma_start(xqb[i][:],
                                          xqb_d[i * 128:(i + 1) * 128, :])

            def proj_g(gk_tiles, src, chunks):
                """gk = M @ x_keys^T, chunk-packed src tile."""
                # jc outer: all chunk-0 groups enter the in-order PE queue
                # first, so they aren't head-of-line blocked behind a
                # chunk-1 group whose DMA lands later
                for jc, ofs, w in chunks:
                    for ct in range(ET):
                        g_ps = ps_s.tile([128, 512], FP32, tag="s", name="g_ps")
                        for kt in range(ET):
                            nc.tensor.matmul(
                                g_ps[:, :w],
                                mt[:, kt * E + ct * 128:kt * E + (ct + 1) * 128],
                                src[:, ofs + kt * w:ofs + (kt + 1) * w],
                                start=(kt == 0), stop=(kt == ET - 1))
                        if fp8:
                            dst = gk_tiles[ct // 2][:, ct % 2:ct % 2 + 1,
                                                    jc * 512:jc * 512 + w]
                        else:
                            dst = gk_tiles[ct][:, jc * 512:jc * 512 + w]
                        nc.vector.tensor_copy(dst, g_ps[:, :w])

            def proj_v(vv_tiles, src, chunks, nj, mofs):
                EV = E + 1
                for jt in range(nj):
                    jc, ofs, w = chunks[(jt * 128) // 512]
                    c0 = jt * 128 - jc * 512
                    v_ps = ps_h.tile([128, EV], FP32, tag="h", name="h_ps")
                    for kt in range(ET):
                        lhsT = src[:, ofs + kt * w + c0:ofs + kt * w + c0 + 128]
                        nc.tensor.matmul(v_ps[:, 0:512], lhsT,
                                         wv[:, kt * EV:kt * EV + 512],
                                         start=(kt == 0), stop=(kt == ET - 1))
                        nc.tensor.matmul(v_ps[:, 512:EV], lhsT,
                                         wv[:, kt * EV + 512:(kt + 1) * EV],
                                         start=(kt == 0), stop=(kt == ET - 1))
                    nc.vector.tensor_add(vv_tiles[jt][:, 0:E], v_ps[:, 0:E],
                                         wb[:, 0:E])
                    nc.vector.tensor_add(ebias[:, mofs + jt:mofs + jt + 1],
                                         v_ps[:, E:E + 1],
                                         mask[:, mofs + jt:mofs + jt + 1])
                    nc.vector.memset(vv_tiles[jt][:, E:E + 1], 1.0)

            def att_qb(qb, tag, gk_tiles, vv_tiles, q_tiles, h_out, nj, mofs,
                       out_eng):
                probs = []
                for jt in range(nj):
                    s_ps = ps_s.tile([128, 512], FP32, tag="s", name="s_ps")
                    if fp8:
                        for cp in range(ETP):
                            nc.tensor.matmul(
                                s_ps[:],
                                gk_tiles[cp][:, :, jt * 128:(jt + 1) * 128],
                                q_tiles[cp][:, :, qb * QB:(qb + 1) * QB],
                                start=(cp == 0), stop=(cp == ETP - 1),
                                perf_mode=DR)
                    else:
                        for ct in range(ET):
                            nc.tensor.matmul(
                                s_ps[:],
                                gk_tiles[ct][:, jt * 128:(jt + 1) * 128],
                                q_tiles[ct][:, qb * QB:(qb + 1) * QB],
                                start=(ct == 0), stop=(ct == ET - 1))
                    p = prob_pool.tile([128, QB], F16, tag=f"p{tag}{jt}",
                                       name=f"p{tag}{jt}")
                    nc.scalar.activation(p[:], s_ps[:], Exp,
                                         bias=ebias[:, mofs + jt:mofs + jt + 1],
                                         scale=inv_sqrt)
                    probs.append(p)
                last = qb == NQB - 1
                ho = hout_pool.tile([128, 4, E + 1], F16, tag=f"ho{tag}",
                                    name=f"ho{tag}")
                for qs in range(QB // 128):
                    h_ps = ps_h.tile([128, E + 1], FP32, tag="h", name="h_ps")
                    for jt in range(nj):
                        lhsT = probs[jt][:, qs * 128:(qs + 1) * 128]
                        nc.tensor.matmul(h_ps[:, 0:512], lhsT,
                                         vv_tiles[jt][:, 0:512],
                                         start=(jt == 0), stop=(jt == nj - 1))
                        nc.tensor.matmul(h_ps[:, 512:E + 1], lhsT,
                                         vv_tiles[jt][:, 512:E + 1],
                                         start=(jt == 0), stop=(jt == nj - 1))
                    nc.vector.tensor_copy(ho[:, qs:qs + 1, :], h_ps[:])
                    if last:
                        # per-qs writes pipeline the final drain with the
                        # remaining compute instead of one big tail DMA;
                        # alternate rings so the last two drain in parallel
                        eng = out_eng if qs % 2 == 0 else (
                            nc.sync if out_eng is nc.scalar else nc.scalar)
                        eng.dma_start(
                            h_out[:, qb * 4 + qs:qb * 4 + qs + 1, :],
                            ho[:, qs:qs + 1, :])
                if not last:
                    out_eng.dma_start(h_out[:, qb * 4:(qb + 1) * 4, :], ho[:])

            # both proj_g phases before proj_v: their inputs arrive first,
            # and the in-order PE queue would otherwise block proj_g B
            # behind proj_v A's wait for wv
            proj_g(gka, xtk, cha)
            if NB:
                proj_g(gkb, xkb, chb)
            proj_v(vva, xtk, cha, NA, 0)
            if NB:
                proj_v(vvb, xkb, chb, NB, NA)
            for qb in range(NQB):
                att_qb(qb, "a", gka, vva, xq, ha_d, NA, 0, nc.scalar)
                if NB:
                    att_qb(qb, "b", gkb, vvb, xqb, hb_d, NB, NA, nc.sync)
    nc.compile()
    return nc


def _get_program(NA: int, NB: int, fp8: bool = USE_FP8):
    key = (NA, NB, fp8)
    if key not in _prog_cache:
        _prog_cache[key] = _build_program(NA, NB, fp8)
    return _prog_cache[key]


def _plan(nj: np.ndarray):
    """Choose (NA, NB) and donor chunk assignment.

    Returns (NA, NB, chunks) where chunks[c] = (sample, tile_ofs, ntiles)
    is core c's secondary assignment (or None)."""
    njmax = int(nj.max())
    total = int(nj.sum())
    best = None
    for njt in range(max(1, (total + NCORES - 1) // NCORES), njmax):
        for na in range(njt - 1, 0, -1):
            nb = njt - na
            if nb > 4:  # SBUF budget guard; fall back to uniform if infeasible
                continue
            slots = sum(-(-max(0, int(x) - na) // nb) for x in nj)
            if slots <= NCORES:
                best = (na, nb)
                break
        if best:
            break
    if best is None:
        return njmax, 0, [None] * NCORES
    na, nb = best
    chunks = []
    for s in range(len(nj)):
        extra = int(nj[s]) - na
        ofs = na
        while extra > 0:
            take = min(nb, extra)
            chunks.append((s, ofs, take))
            ofs += take
            extra -= take
    chunks += [None] * (NCORES - len(chunks))
    return na, nb, chunks


def _pack_keys(xT: np.ndarray, kcols: int) -> np.ndarray:
    """[E, >=kcols] fp-any -> chunk-packed [128, ET*kcols] fp16."""
    out = np.empty((128, ET * kcols), dtype=F16NP)
    for jc, ofs, w in _chunks_of(kcols):
        blk = xT[:, jc * 512:jc * 512 + w]            # [E, w]
        out[:, ofs:ofs + ET * w] = (
            blk.reshape(ET, 128, w).transpose(1, 0, 2).reshape(128, ET * w))
    return out


def _pack6(a: np.ndarray) -> np.ndarray:
    """[ET*128, C] -> [128, ET*C] fp16 (k-tiles side by side)."""
    C = a.shape[1]
    return np.ascontiguousarray(
        a.reshape(ET, 128, C).transpose(1, 0, 2).reshape(128, ET * C)
    ).astype(F16NP)


def _pack_fp8(xT: np.ndarray, f8np) -> np.ndarray:
    """[E, S] -> DoubleRow-packed [ETP*128, 2, S] fp8."""
    q = np.asarray(xT, dtype=np.float32).astype(f8np)
    return np.ascontiguousarray(
        q.reshape(ETP, 2, 128, S).transpose(0, 2, 1, 3).reshape(ETP * 128, 2, S))


def _prepare_inputs(full_ebd, SEQ_idxes, Wq_w, Wq_b, Wk_w, Wk_b, Wv_w, Wv_b,
                    fp8: bool = USE_FP8):
    from concourse import mybir
    f8np = mybir.dt.np(mybir.dt.float8e4)

    full_ebd = np.asarray(full_ebd, dtype=np.float32)
    first = np.asarray(SEQ_idxes)[:, 0].astype(np.int64)
    nj = np.maximum(1, np.minimum(MAX_NJ, (first + 127) // 128))
    NA, NB, chunks = _plan(nj)
    KA, KB = NA * 128, NB * 128

    Wq64 = np.asarray(Wq_w, dtype=np.float64)
    Wk64 = np.asarray(Wk_w, dtype=np.float64)
    # lhsT for G: mT[k, c] = M[c, k],  M = Wq^T Wk
    mT = (Wk64.T @ Wq64).astype(F16NP)
    # packed ct-major: [p, ct*E + kt*128 + cc] = mT[kt*128+p, ct*128+cc],
    # so proj_g's first output block only needs the leading E columns
    mt_ctmaj = np.ascontiguousarray(
        mT.reshape(ET, 128, ET, 128).transpose(1, 2, 0, 3).reshape(128, ET * E))
    v = (Wk64.T @ np.asarray(Wq_b, dtype=np.float64)) / np.sqrt(E)
    wv_aug = np.zeros((E, E + 1), dtype=np.float64)
    wv_aug[:, 0:E] = np.asarray(Wv_w, dtype=np.float64).T
    wv_aug[:, E] = v
    wb_row = np.zeros((E + 1,), dtype=np.float64)
    wb_row[0:E] = np.asarray(Wv_b, dtype=np.float64)
    wb_f16 = np.ascontiguousarray(np.broadcast_to(wb_row, (128, E + 1))).astype(F16NP)

    mt_p = _pack6(mT)
    wv_p = _pack6(wv_aug.astype(F16NP))

    def make_mask(sample, tile_ofs, ntiles):
        j = tile_ofs * 128 + np.arange(ntiles * 128)
        valid = (j >= 1) & (j < first[sample])
        m = np.where(valid, 0.0, -300.0).astype(np.float32)
        return np.ascontiguousarray(m.reshape(ntiles, 128).T)

    xts = [np.ascontiguousarray(full_ebd[b].T) for b in range(B)]
    xts16 = [x.astype(F16NP) for x in xts]
    if fp8:
        xq8s = [_pack_fp8(x, f8np) for x in xts]
    in_maps = []
    for c in range(NCORES):
        maskab = np.full((128, NA + NB), -300.0, dtype=np.float32)
        maskab[:, :NA] = make_mask(c, 0, NA)
        im = {"xtk": _pack_keys(xts16[c], KA), "mt": mt_p,
              "wv": wv_p, "wb": wb_f16}
        if fp8:
            im["xq8"] = xq8s[c]
        else:
            im["xq"] = xts16[c]
        if NB:
            if chunks[c] is not None:
                s, ofs, take = chunks[c]
                xkb = np.zeros((E, KB), dtype=F16NP)
                xkb[:, :take * 128] = xts16[s][:, ofs * 128:(ofs + take) * 128]
                im["xkb"] = _pack_keys(xkb, KB)
                maskab[:, NA:NA + take] = make_mask(s, ofs, take)
                if fp8:
                    im["xq8b"] = xq8s[s]
                else:
                    im["xqb"] = xts16[s]
            else:
                im["xkb"] = np.zeros((128, ET * KB), dtype=F16NP)
                if fp8:
                    im["xq8b"] = xq8s[c]
                else:
                    im["xqb"] = xts16[c]
        im["mask"] = maskab
        in_maps.append(im)
    return (NA, NB, chunks), in_maps


def _unscramble(h):
    """[128, S/128, E+1] -> [S, E+1] float64."""
    h = np.asarray(h, dtype=np.float64)
    return h.transpose(1, 0, 2).reshape(S, E + 1)


def _combine(results, plan):
    NA, NB, chunks = plan
    out = np.empty((B, S, E), dtype=np.float32)
    for s in range(B):
        acc = _unscramble(results[s]["ha"])
        if NB:
            for c in range(NCORES):
                if chunks[c] is not None and chunks[c][0] == s:
                    acc = acc + _unscramble(results[c]["hb"])
        out[s] = (acc[:, :E] / acc[:, E:E + 1]).astype(np.float32)
    return out


def _run(in_maps, plan, fp8: bool = USE_FP8, **kwargs):
    from concourse.bass_utils import run_bass_kernel_spmd

    nc = _get_program(plan[0], plan[1], fp8)
    return run_bass_kernel_spmd(nc, in_maps, core_ids=list(range(NCORES)), **kwargs)


def kernel(full_ebd, SEQ_idxes, Wq_w, Wq_b, Wk_w, Wk_b, Wv_w, Wv_b):
    plan, in_maps = _prepare_inputs(full_ebd, SEQ_idxes, Wq_w, Wq_b,
                                    Wk_w, Wk_b, Wv_w, Wv_b)
    res = _run(in_maps, plan)
    return _combine(res.results, plan)
